# revision 4
# baseline (speedup 1.0000x reference)
"""BitLinear TRN2 kernel, fp8e4 DoubleRow design.

y[b,m] = (q @ unpack2bit(W).T)[b,m] * (gs/s)[b],  q = int8-quantized input.

Column-parallel over 8 cores (1376 of 11008 out features each).

Device algorithm per core:
  - Packed weight bytes stream from HBM as int16 words (2 bytes/elem),
    1.41 MB/core -- the information-theoretic minimum (2 bits/weight).
  - DVE builds 4 "t-planes" per 128-row j-tile with ONE tensor_scalar each:
    plane_r = (word >> 2r) & 0x0303 extracts the r-th 2-bit field of BOTH
    packed bytes per word.  The resulting bytes 0..3, bitcast to fp8e4, read
    as exact values t * 2^-9 (denormals).
  - PE runs DoubleRow fp8 matmuls (256-wide contraction, ~1 cycle/output
    col): rhs = [plane_2p | plane_2p+1] (pair adjacent in free dim), lhsT
    [128,2,64] holds per-plane coefficients [16*qh | 0 | ql | 0] where
    q = 16*qh + ql splits the int8 activations into two exactly-fp8-
    representable halves.  PSUM rows 0:16 accumulate 2^-9 * sum 16qh*t,
    rows 32:48 the ql part (32-aligned for legal partition access) --
    all arithmetic is exact.
  - Matmuls are emitted in bursts of 2-4 groups per PSUM bank to limit
    accumulate-group transitions (PE micro-idles / clock re-throttle).
  - Epilogue: osb = psum_hi*c0 + (psum_lo*c0 - c1) with c0 = 512*gs/s,
    c1 = S_q*gs/s (since sum q*t = y_int + S_q), scale ops split across
    the Act and DVE engines, adds on DVE.

Optionally some planes are pre-expanded host-side and DMAd as fp8 directly
(HOST_JT j-tiles) to rebalance DVE vs DMA time (off by default).
"""

import os
import sys

sys.path.insert(0, "/opt/trn_rl_repo")

import numpy as np

import concourse.mybir as mybir
import concourse.tile as tile
from concourse import bacc
from concourse.bass_utils import run_bass_kernel_spmd

AluOp = mybir.AluOpType
PM = mybir.MatmulPerfMode
f32 = mybir.dt.float32
f16 = mybir.dt.float16
f8 = mybir.dt.float8e4
i16 = mybir.dt.int16
i32 = mybir.dt.int32

B = 16
K = 4096
M = 11008
KP = K // 4          # 1024 packed bytes per output row
NCORES = 8
MS = M // NCORES     # 1376
NJT = KP // 128      # 8 j-tiles per core
CHUNKS = [(0, 512), (512, 512), (1024, MS - 1024)]
NCONST = 3

KVAR = os.environ.get("KVAR", "base")
N_WARMUP = int(os.environ.get("NWARM", "8"))
# j-tiles whose 4 planes are host-expanded to fp8 and DMAd (no DVE work)
N_HOST_JT = int(os.environ.get("NHOSTJT", "0"))
HOST_JT = list(range(NJT - N_HOST_JT, NJT))


def build_kernel_body(tc, wb_d, coef_d, consts_d, hpl_d, out_d):
    nc = tc.nc
    with (
        tc.tile_pool(name="sbuf", bufs=1) as pool,
        tc.tile_pool(name="psum", bufs=1, space="PSUM") as psum_pool,
    ):
        psums = [
            psum_pool.tile([64, ln], f32, tag=f"psum{ci}", name=f"psum{ci}")
            for ci, (_, ln) in enumerate(CHUNKS)
        ]

        # memsets for PE warmup data go FIRST on gpsimd (engine op, before
        # its DMA-issue work) so warmups can start in the head dead zone
        warm_l = pool.tile([128, 2, 64], f8, tag="warml")
        warm_r = pool.tile([128, 2, 512], f8, tag="warmr")
        nc.gpsimd.memset(warm_l[:], 1.0)
        nc.gpsimd.memset(warm_r[:], 1.0)

        # wb0 is the very first transfer (gates the first DVE plane op);
        # coef goes first on the scalar queue (needed by first real matmul)
        coef = pool.tile([128, NJT, 2, 2, 64], f8, tag="coef")
        consts = pool.tile([B, NCONST], f32, tag="consts")
        queues = [nc.sync, nc.scalar, nc.gpsimd]
        qassign = {0: nc.sync, 1: nc.scalar, 2: nc.gpsimd, 3: nc.sync,
                   4: nc.scalar, 5: nc.gpsimd, 6: nc.sync, 7: nc.scalar}
        wbs = {}
        hpls = {}
        wb_tiles = {}
        for j in range(NJT):
            if j in HOST_JT:
                hp = pool.tile(
                    [128, 2, 2, MS], f8, tag=f"hpl{j}", name=f"hpl{j}"
                )
                hpls[j] = hp
            else:
                wb = pool.tile(
                    [128, MS // 2], i16, tag=f"wb{j}", name=f"wb{j}"
                )
                wbs[j] = wb
        if 0 not in HOST_JT:
            # wb0 gates the whole pipeline: halves on the 2 HWDGE queues
            h = MS // 4
            nc.sync.dma_start(wbs[0][:, 0:h], wb_d[0][:, 0:h])
            nc.scalar.dma_start(wbs[0][:, h:], wb_d[0][:, h:])
        else:
            nc.sync.dma_start(hpls[0][:, 0], hpl_d[0][:, 0])
            nc.scalar.dma_start(hpls[0][:, 1], hpl_d[0][:, 1])
        nc.scalar.dma_start(coef[:], coef_d[:])
        nc.sync.dma_start(consts[:], consts_d[:])
        # gpsimd is SWDGE (slow issue + heavy end-of-kernel drain): avoid it
        qassign2 = {1: nc.sync, 2: nc.scalar, 3: nc.sync, 4: nc.scalar,
                    5: nc.sync, 6: nc.scalar, 7: nc.sync}
        for j in range(1, NJT):
            q = qassign2[j]
            if j in HOST_JT:
                q.dma_start(hpls[j][:, 0], hpl_d[j][:, 0])
                q.dma_start(hpls[j][:, 1], hpl_d[j][:, 1])
            else:
                q.dma_start(wbs[j][:], wb_d[j][:])

        # pay the one-time Act table load early, off the critical path
        # (Identity is the table the epilogue uses)
        actwarm = pool.tile([B, NCONST], f32, tag="actwarm")
        nc.scalar.activation(
            actwarm[:], consts[:], mybir.ActivationFunctionType.Identity
        )

        # PE warmup in the DMA dead zone
        warm_ps = psum_pool.tile([64, 512], f32, tag="warmps")
        for _ in range(N_WARMUP):
            nc.tensor.matmul(
                warm_ps[:], warm_l[:], warm_r[:],
                start=True, stop=True, perf_mode=PM.DoubleRow,
            )

        # build planes + matmul, j outer
        planes = {}
        for j in range(NJT):
            if j in HOST_JT:
                continue
            wb = wbs[j]
            pls = [
                pool.tile(
                    [128, 2, MS // 2], i16, tag=f"pl{j}_{pr}", name=f"pl{j}_{pr}"
                )
                for pr in range(2)
            ]
            for pr in range(2):
                for s in range(2):
                    r = 2 * pr + s
                    nc.vector.tensor_scalar(
                        pls[pr][:, s, :], wb[:], 2 * r, 0x0303,
                        AluOp.logical_shift_right, AluOp.bitwise_and,
                    )
            planes[j] = pls

        # group-outer for steady-state pipelining; the last 3 groups are
        # re-ordered chunk-outer so the chunk stops fire staggered and the
        # epilogues overlap the remaining matmuls
        def rhs_of(j, pr):
            if j in HOST_JT:
                return hpls[j][:, pr]
            return planes[j][pr][:].bitcast(f8)

        groups = [(j, pr) for j in range(NJT) for pr in range(2)]
        # bursts: several groups per PSUM bank before switching (fewer
        # accumulate-group transitions -> less PE micro-idling), and the
        # last burst's chunk-0 run finishes well before the stream end so
        # the epilogues pipeline into the stream
        bursts = [groups[0:2], groups[2:4], groups[4:8], groups[8:12], groups[12:16]]
        # chunks 0/1 complete first; chunk 2 runs as one final 16-matmul
        # block, so the chunk-0/1 epilogues overlap it on Act/DVE
        emit = [
            (g, ci)
            for burst in bursts
            for ci in (0, 1)
            for g in burst
        ]
        emit += [(g, 2) for g in groups]
        for (j, pr), ci in emit:
            off, ln = CHUNKS[ci]
            nc.tensor.matmul(
                psums[ci][:],
                coef[:, j, pr, :, :],
                rhs_of(j, pr)[:, :, off : off + ln],
                start=(j == 0 and pr == 0),
                stop=(j == NJT - 1 and pr == 1),
                perf_mode=PM.DoubleRow,
            )

        # epilogue: osb = psum_hi*c0 + (psum_lo*c0 - c1).  Chunks 0/1 are
        # finished completely (scales, add, store) BEFORE any chunk-2 op is
        # queued, so their DVE work overlaps the trailing chunk-2 matmul
        # block instead of stalling behind its psum wait (in-order engines).
        store_engines = [nc.sync, nc.scalar]
        for ci, (off, ln) in enumerate(CHUNKS):
            u = pool.tile([B, ln], f32, tag=f"u{ci}", name=f"u{ci}")
            v = pool.tile([B, ln], f32, tag=f"v{ci}", name=f"v{ci}")
            if ci == 0:
                # Act: u = Identity(lo*c0 + (-c1)), v = Copy(hi*c0)
                nc.scalar.activation(
                    u[:], psums[ci][32:48, :],
                    mybir.ActivationFunctionType.Identity,
                    bias=consts[:, 2:3],
                    scale=consts[:, 0:1],
                )
                nc.scalar.activation(
                    v[:], psums[ci][0:16, :],
                    mybir.ActivationFunctionType.Copy,
                    scale=consts[:, 0:1],
                )
            elif ci == 1:
                nc.vector.tensor_scalar(
                    u[:], psums[ci][32:48, :], consts[:, 0:1], consts[:, 1:2],
                    AluOp.mult, AluOp.subtract,
                )
                nc.scalar.activation(
                    v[:], psums[ci][0:16, :],
                    mybir.ActivationFunctionType.Copy,
                    scale=consts[:, 0:1],
                )
            else:
                nc.vector.tensor_scalar(
                    u[:], psums[ci][32:48, :], consts[:, 0:1], consts[:, 1:2],
                    AluOp.mult, AluOp.subtract,
                )
                nc.scalar.activation(
                    v[:], psums[ci][0:16, :],
                    mybir.ActivationFunctionType.Copy,
                    scale=consts[:, 0:1],
                )
            osb = pool.tile([B, ln], f32, tag=f"osb{ci}", name=f"osb{ci}")
            nc.vector.tensor_tensor(osb[:], v[:], u[:], AluOp.add)
            store_engines[ci % 2].dma_start(out_d[:, off : off + ln], osb[:])


def build_nc():
    nc = bacc.Bacc("TRN2", target_bir_lowering=False)
    wb_d = [
        nc.dram_tensor(f"wb{j}", [128, MS // 2], i16, kind="ExternalInput")
        for j in range(NJT)
        if j not in HOST_JT
    ]
    wb_map = {}
    k = 0
    for j in range(NJT):
        if j not in HOST_JT:
            wb_map[j] = wb_d[k]
            k += 1
    coef_d = nc.dram_tensor(
        "coef", [128, NJT, 2, 2, 64], f8, kind="ExternalInput"
    )
    consts_d = nc.dram_tensor("consts", [B, NCONST], f32, kind="ExternalInput")
    hpl_map = {}
    for j in HOST_JT:
        hpl_map[j] = nc.dram_tensor(
            f"hpl{j}", [128, 2, 2, MS], f8, kind="ExternalInput"
        )
    out_d = nc.dram_tensor("out", [B, MS], f32, kind="ExternalOutput")
    with tile.TileContext(nc) as tc:
        build_kernel_body(tc, wb_map, coef_d, consts_d, hpl_map, out_d)
    nc.compile()
    return nc


def prepare_inputs(input, weight_packed, weight_scale):
    inp = np.asarray(input, dtype=np.float32)
    wp = np.asarray(weight_packed, dtype=np.int32)
    ws = np.asarray(weight_scale, dtype=np.float32)
    np8 = mybir.dt.np(f8)

    # activation quantization (matches reference)
    amax = np.maximum(np.max(np.abs(inp), axis=-1, keepdims=True), np.float32(1e-5))
    s = np.float32(127.0) / amax
    q = np.clip(np.round(inp * s), -128.0, 127.0).astype(np.float32)  # [B,K]

    # split q = 16*qh + ql, both exactly fp8e4-representable
    qh = np.round(q / 16.0)
    ql = q - 16.0 * qh

    # coef[kp_part, j, pr, s, 0:16] = 16*qh[b, 4*(128j+kp)+2pr+s]
    #                        [16:32] = ql[...]
    qr = q.reshape(B, KP, 4)
    qhr = qh.reshape(B, KP, 4)
    qlr = ql.reshape(B, KP, 4)
    coef = np.zeros((128, NJT, 2, 2, 64), dtype=np8)
    kp_idx = np.arange(KP).reshape(NJT, 128)
    for j in range(NJT):
        for pr in range(2):
            for sbi in range(2):
                r = 2 * pr + sbi
                coef[:, j, pr, sbi, 0:16] = (
                    16.0 * qhr[:, kp_idx[j], r]
                ).T.astype(np8)
                coef[:, j, pr, sbi, 32:48] = qlr[:, kp_idx[j], r].T.astype(np8)

    S_q = q.sum(axis=-1)  # [B]
    in_maps = []
    for core in range(NCORES):
        m0 = core * MS
        gs = ws[m0 // (M // ws.shape[0])]
        # byte tile per j-tile: [128 kp, MS m] u8 -> i16 words (2 m per word)
        w8 = np.ascontiguousarray(wp[m0 : m0 + MS].astype(np.uint8).T)  # [KP, MS]
        im = {"coef": coef}
        consts = np.zeros((B, 3), dtype=np.float32)
        consts[:, 0] = np.float32(512.0) * gs / s[:, 0]
        consts[:, 1] = S_q * gs / s[:, 0]
        consts[:, 2] = -consts[:, 1]
        im["consts"] = consts
        for j in range(NJT):
            blk = w8[128 * j : 128 * (j + 1)]  # [128, MS] u8
            if j in HOST_JT:
                t = np.stack(
                    [(blk >> (2 * r)) & 3 for r in range(4)], axis=1
                )  # [128, 4, MS] u8: planes as raw fp8 bit patterns
                im[f"hpl{j}"] = t.reshape(128, 2, 2, MS).view(np8)
            else:
                im[f"wb{j}"] = np.ascontiguousarray(blk).view("<i2")
        in_maps.append(im)
    return in_maps


_NC_CACHE = {}


def run(input, weight_packed, weight_scale, trace=False):
    if "nc" not in _NC_CACHE:
        _NC_CACHE["nc"] = build_nc()
    nc = _NC_CACHE["nc"]
    in_maps = prepare_inputs(input, weight_packed, weight_scale)
    res = run_bass_kernel_spmd(nc, in_maps, core_ids=list(range(NCORES)), trace=trace)
    out = np.concatenate([r["out"] for r in res.results], axis=1)
    return out, res


def kernel(**inputs):
    out, _ = run(
        inputs["input"], inputs["weight_packed"], inputs["weight_scale"], trace=False
    )
    return out


# revision 5
# speedup vs baseline: 1.0113x; 1.0113x over previous
"""BitLinear TRN2 kernel, fp8e4 DoubleRow design.

y[b,m] = (q @ unpack2bit(W).T)[b,m] * (gs/s)[b],  q = int8-quantized input.

Column-parallel over 8 cores (1376 of 11008 out features each).

Device algorithm per core:
  - Packed weight bytes stream from HBM as int16 words (2 bytes/elem),
    1.41 MB/core -- the information-theoretic minimum (2 bits/weight).
  - DVE builds 4 "t-planes" per 128-row j-tile with ONE tensor_scalar each:
    plane_r = (word >> 2r) & 0x0303 extracts the r-th 2-bit field of BOTH
    packed bytes per word.  The resulting bytes 0..3, bitcast to fp8e4, read
    as exact values t * 2^-9 (denormals).
  - PE runs DoubleRow fp8 matmuls (256-wide contraction, ~1 cycle/output
    col): rhs = [plane_2p | plane_2p+1] (pair adjacent in free dim), lhsT
    [128,2,64] holds per-plane coefficients [16*qh | 0 | ql | 0] where
    q = 16*qh + ql splits the int8 activations into two exactly-fp8-
    representable halves.  PSUM rows 0:16 accumulate 2^-9 * sum 16qh*t,
    rows 32:48 the ql part (32-aligned for legal partition access) --
    all arithmetic is exact.
  - Matmuls are emitted in bursts of 2-4 groups per PSUM bank to limit
    accumulate-group transitions (PE micro-idles / clock re-throttle).
  - Epilogue: osb = psum_hi*c0 + (psum_lo*c0 - c1) with c0 = 512*gs/s,
    c1 = S_q*gs/s (since sum q*t = y_int + S_q), scale ops split across
    the Act and DVE engines, adds on DVE.

Optionally some planes are pre-expanded host-side and DMAd as fp8 directly
(HOST_JT j-tiles) to rebalance DVE vs DMA time (off by default).
"""

import os
import sys

sys.path.insert(0, "/opt/trn_rl_repo")

import numpy as np

import concourse.mybir as mybir
import concourse.tile as tile
from concourse import bacc
from concourse.bass_utils import run_bass_kernel_spmd

AluOp = mybir.AluOpType
PM = mybir.MatmulPerfMode
f32 = mybir.dt.float32
f16 = mybir.dt.float16
f8 = mybir.dt.float8e4
i16 = mybir.dt.int16
i32 = mybir.dt.int32

B = 16
K = 4096
M = 11008
KP = K // 4          # 1024 packed bytes per output row
NCORES = 8
MS = M // NCORES     # 1376
NJT = KP // 128      # 8 j-tiles per core
CHUNKS = [(0, 512), (512, 512), (1024, MS - 1024)]
NCONST = 3

KVAR = os.environ.get("KVAR", "base")
N_WARMUP = int(os.environ.get("NWARM", "8"))
# j-tiles whose 4 planes are host-expanded to fp8 and DMAd (no DVE work)
N_HOST_JT = int(os.environ.get("NHOSTJT", "0"))
HOST_JT = list(range(NJT - N_HOST_JT, NJT))


def build_kernel_body(tc, wb_d, coef_d, consts_d, hpl_d, out_d):
    nc = tc.nc
    with (
        tc.tile_pool(name="sbuf", bufs=1) as pool,
        tc.tile_pool(name="psum", bufs=1, space="PSUM") as psum_pool,
    ):
        psums = [
            psum_pool.tile([64, ln], f32, tag=f"psum{ci}", name=f"psum{ci}")
            for ci, (_, ln) in enumerate(CHUNKS)
        ]

        # memsets for PE warmup data go FIRST on gpsimd (engine op, before
        # its DMA-issue work) so warmups can start in the head dead zone
        warm_l = pool.tile([128, 2, 64], f8, tag="warml")
        warm_r = pool.tile([128, 2, 512], f8, tag="warmr")
        nc.gpsimd.memset(warm_l[:], 1.0)
        nc.gpsimd.memset(warm_r[:], 1.0)

        # wb0 is the very first transfer (gates the first DVE plane op);
        # coef goes first on the scalar queue (needed by first real matmul)
        coef = pool.tile([128, NJT, 2, 2, 64], f8, tag="coef")
        consts = pool.tile([B, NCONST], f32, tag="consts")
        queues = [nc.sync, nc.scalar, nc.gpsimd]
        qassign = {0: nc.sync, 1: nc.scalar, 2: nc.gpsimd, 3: nc.sync,
                   4: nc.scalar, 5: nc.gpsimd, 6: nc.sync, 7: nc.scalar}
        wbs = {}
        hpls = {}
        wb_tiles = {}
        for j in range(NJT):
            if j in HOST_JT:
                hp = pool.tile(
                    [128, 2, 2, MS], f8, tag=f"hpl{j}", name=f"hpl{j}"
                )
                hpls[j] = hp
            else:
                wb = pool.tile(
                    [128, MS // 2], i16, tag=f"wb{j}", name=f"wb{j}"
                )
                wbs[j] = wb
        if 0 not in HOST_JT:
            # wb0 gates the whole pipeline: halves on the 2 HWDGE queues
            h = MS // 4
            nc.sync.dma_start(wbs[0][:, 0:h], wb_d[0][:, 0:h])
            nc.scalar.dma_start(wbs[0][:, h:], wb_d[0][:, h:])
        else:
            nc.sync.dma_start(hpls[0][:, 0], hpl_d[0][:, 0])
            nc.scalar.dma_start(hpls[0][:, 1], hpl_d[0][:, 1])
        nc.scalar.dma_start(coef[:], coef_d[:])
        nc.sync.dma_start(consts[:], consts_d[:])
        # gpsimd is SWDGE (slow issue + heavy end-of-kernel drain): avoid it
        qassign2 = {1: nc.sync, 2: nc.scalar, 3: nc.sync, 4: nc.scalar,
                    5: nc.sync, 6: nc.scalar, 7: nc.sync}
        for j in range(1, NJT):
            q = qassign2[j]
            if j in HOST_JT:
                q.dma_start(hpls[j][:, 0], hpl_d[j][:, 0])
                q.dma_start(hpls[j][:, 1], hpl_d[j][:, 1])
            else:
                q.dma_start(wbs[j][:], wb_d[j][:])

        # pay the one-time Act table load early, off the critical path
        # (Identity is the table the epilogue uses)
        actwarm = pool.tile([B, NCONST], f32, tag="actwarm")
        nc.scalar.activation(
            actwarm[:], consts[:], mybir.ActivationFunctionType.Identity
        )

        # PE warmup in the DMA dead zone
        warm_ps = psum_pool.tile([64, 512], f32, tag="warmps")
        for _ in range(N_WARMUP):
            nc.tensor.matmul(
                warm_ps[:], warm_l[:], warm_r[:],
                start=True, stop=True, perf_mode=PM.DoubleRow,
            )

        # build planes + matmul, j outer
        planes = {}
        for j in range(NJT):
            if j in HOST_JT:
                continue
            wb = wbs[j]
            pls = [
                pool.tile(
                    [128, 2, MS // 2], i16, tag=f"pl{j}_{pr}", name=f"pl{j}_{pr}"
                )
                for pr in range(2)
            ]
            for pr in range(2):
                for s in range(2):
                    r = 2 * pr + s
                    nc.vector.tensor_scalar(
                        pls[pr][:, s, :], wb[:], 2 * r, 0x0303,
                        AluOp.logical_shift_right, AluOp.bitwise_and,
                    )
            planes[j] = pls

        # group-outer for steady-state pipelining; the last 3 groups are
        # re-ordered chunk-outer so the chunk stops fire staggered and the
        # epilogues overlap the remaining matmuls
        def rhs_of(j, pr):
            if j in HOST_JT:
                return hpls[j][:, pr]
            return planes[j][pr][:].bitcast(f8)

        groups = [(j, pr) for j in range(NJT) for pr in range(2)]
        # bursts: several groups per PSUM bank before switching (fewer
        # accumulate-group transitions -> less PE micro-idling)
        bursts = [groups[0:2], groups[2:4], groups[4:8], groups[8:12], groups[12:16]]
        # chunks 0/1 complete first; chunk 2 runs as one final 16-matmul
        # block, so the chunk-0/1 epilogues overlap it on Act/DVE
        emit = [
            (g, ci)
            for burst in bursts
            for ci in (0, 1)
            for g in burst
        ]
        emit += [(g, 2) for g in groups]
        for (j, pr), ci in emit:
            off, ln = CHUNKS[ci]
            nc.tensor.matmul(
                psums[ci][:],
                coef[:, j, pr, :, :],
                rhs_of(j, pr)[:, :, off : off + ln],
                start=(j == 0 and pr == 0),
                stop=(j == NJT - 1 and pr == 1),
                perf_mode=PM.DoubleRow,
            )

        # epilogue: osb = psum_hi*c0 + (psum_lo*c0 - c1).  Chunks 0/1 are
        # finished completely (scales, add, store) BEFORE any chunk-2 op is
        # queued, so their DVE work overlaps the trailing chunk-2 matmul
        # block instead of stalling behind its psum wait (in-order engines).
        store_engines = [nc.sync, nc.scalar]
        for ci, (off, ln) in enumerate(CHUNKS):
            u = pool.tile([B, ln], f32, tag=f"u{ci}", name=f"u{ci}")
            v = pool.tile([B, ln], f32, tag=f"v{ci}", name=f"v{ci}")
            if ci == 0:
                # Act: u = Identity(lo*c0 + (-c1)), v = Copy(hi*c0)
                nc.scalar.activation(
                    u[:], psums[ci][32:48, :],
                    mybir.ActivationFunctionType.Identity,
                    bias=consts[:, 2:3],
                    scale=consts[:, 0:1],
                )
                nc.scalar.activation(
                    v[:], psums[ci][0:16, :],
                    mybir.ActivationFunctionType.Copy,
                    scale=consts[:, 0:1],
                )
            elif ci == 1:
                nc.vector.tensor_scalar(
                    u[:], psums[ci][32:48, :], consts[:, 0:1], consts[:, 1:2],
                    AluOp.mult, AluOp.subtract,
                )
                nc.scalar.activation(
                    v[:], psums[ci][0:16, :],
                    mybir.ActivationFunctionType.Copy,
                    scale=consts[:, 0:1],
                )
            else:
                nc.vector.tensor_scalar(
                    u[:], psums[ci][32:48, :], consts[:, 0:1], consts[:, 1:2],
                    AluOp.mult, AluOp.subtract,
                )
                nc.scalar.activation(
                    v[:], psums[ci][0:16, :],
                    mybir.ActivationFunctionType.Copy,
                    scale=consts[:, 0:1],
                )
            osb = pool.tile([B, ln], f32, tag=f"osb{ci}", name=f"osb{ci}")
            nc.vector.tensor_tensor(osb[:], v[:], u[:], AluOp.add)
            store_engines[ci % 2].dma_start(out_d[:, off : off + ln], osb[:])


def build_nc():
    nc = bacc.Bacc("TRN2", target_bir_lowering=False)
    wb_d = [
        nc.dram_tensor(f"wb{j}", [128, MS // 2], i16, kind="ExternalInput")
        for j in range(NJT)
        if j not in HOST_JT
    ]
    wb_map = {}
    k = 0
    for j in range(NJT):
        if j not in HOST_JT:
            wb_map[j] = wb_d[k]
            k += 1
    coef_d = nc.dram_tensor(
        "coef", [128, NJT, 2, 2, 64], f8, kind="ExternalInput"
    )
    consts_d = nc.dram_tensor("consts", [B, NCONST], f32, kind="ExternalInput")
    hpl_map = {}
    for j in HOST_JT:
        hpl_map[j] = nc.dram_tensor(
            f"hpl{j}", [128, 2, 2, MS], f8, kind="ExternalInput"
        )
    out_d = nc.dram_tensor("out", [B, MS], f32, kind="ExternalOutput")
    with tile.TileContext(nc) as tc:
        build_kernel_body(tc, wb_map, coef_d, consts_d, hpl_map, out_d)
    nc.compile()
    return nc


def prepare_inputs(input, weight_packed, weight_scale):
    inp = np.asarray(input, dtype=np.float32)
    wp = np.asarray(weight_packed, dtype=np.int32)
    ws = np.asarray(weight_scale, dtype=np.float32)
    np8 = mybir.dt.np(f8)

    # activation quantization (matches reference)
    amax = np.maximum(np.max(np.abs(inp), axis=-1, keepdims=True), np.float32(1e-5))
    s = np.float32(127.0) / amax
    q = np.clip(np.round(inp * s), -128.0, 127.0).astype(np.float32)  # [B,K]

    # split q = 16*qh + ql, both exactly fp8e4-representable
    qh = np.round(q / 16.0)
    ql = q - 16.0 * qh

    # coef[kp_part, j, pr, s, 0:16] = 16*qh[b, 4*(128j+kp)+2pr+s]
    #                        [16:32] = ql[...]
    qr = q.reshape(B, KP, 4)
    qhr = qh.reshape(B, KP, 4)
    qlr = ql.reshape(B, KP, 4)
    coef = np.zeros((128, NJT, 2, 2, 64), dtype=np8)
    kp_idx = np.arange(KP).reshape(NJT, 128)
    for j in range(NJT):
        for pr in range(2):
            for sbi in range(2):
                r = 2 * pr + sbi
                coef[:, j, pr, sbi, 0:16] = (
                    16.0 * qhr[:, kp_idx[j], r]
                ).T.astype(np8)
                coef[:, j, pr, sbi, 32:48] = qlr[:, kp_idx[j], r].T.astype(np8)

    S_q = q.sum(axis=-1)  # [B]
    in_maps = []
    for core in range(NCORES):
        m0 = core * MS
        gs = ws[m0 // (M // ws.shape[0])]
        # byte tile per j-tile: [128 kp, MS m] u8 -> i16 words (2 m per word)
        w8 = np.ascontiguousarray(wp[m0 : m0 + MS].astype(np.uint8).T)  # [KP, MS]
        im = {"coef": coef}
        consts = np.zeros((B, 3), dtype=np.float32)
        consts[:, 0] = np.float32(512.0) * gs / s[:, 0]
        consts[:, 1] = S_q * gs / s[:, 0]
        consts[:, 2] = -consts[:, 1]
        im["consts"] = consts
        for j in range(NJT):
            blk = w8[128 * j : 128 * (j + 1)]  # [128, MS] u8
            if j in HOST_JT:
                t = np.stack(
                    [(blk >> (2 * r)) & 3 for r in range(4)], axis=1
                )  # [128, 4, MS] u8: planes as raw fp8 bit patterns
                im[f"hpl{j}"] = t.reshape(128, 2, 2, MS).view(np8)
            else:
                im[f"wb{j}"] = np.ascontiguousarray(blk).view("<i2")
        in_maps.append(im)
    return in_maps


_NC_CACHE = {}


def run(input, weight_packed, weight_scale, trace=False):
    if "nc" not in _NC_CACHE:
        _NC_CACHE["nc"] = build_nc()
    nc = _NC_CACHE["nc"]
    in_maps = prepare_inputs(input, weight_packed, weight_scale)
    res = run_bass_kernel_spmd(nc, in_maps, core_ids=list(range(NCORES)), trace=trace)
    out = np.concatenate([r["out"] for r in res.results], axis=1)
    return out, res


def kernel(**inputs):
    out, _ = run(
        inputs["input"], inputs["weight_packed"], inputs["weight_scale"], trace=False
    )
    return out
